# revision 2
# baseline (speedup 1.0000x reference)
"""Block floating-point quantization (block=16 along last dim, 8 mantissa bits)
for x of shape (4, 4096, 4096) f32, distributed over 8 NeuronCores.

Per 16-element block along the last dim:
  step = 2^(floor(log2(max|x|)) - 7);  q = clip(round(x/step), -128, 127) * step

Implementation per core-shard tile [128, 4096]:
  DVE:    absmax-reduce over blocks; bit-tricks for step/recip; scale x*recip
  ACT:    f32 -> i8 convert (RNE + saturate == round + clip)
  GPSIMD: dequant q8 * step (block-broadcast)
Sharding: x flattened to (16384, 4096); core c takes rows [2048c, 2048(c+1)).
"""
import numpy as np

import concourse.bacc as bacc
import concourse.mybir as mybir
from concourse.tile import TileContext
from concourse.bass_utils import run_bass_kernel_spmd

N_CORES = 8
FULL_SHAPE = (4, 4096, 4096)
ROWS, COLS = 16384, 4096  # flattened
SH_ROWS = ROWS // N_CORES  # 2048 rows per core
BLK = 16
TILE_P = 128
TILE_F = 4096
N_TILES = SH_ROWS // TILE_P  # 16
NB = TILE_F // BLK  # 256 blocks per partition-row

F32 = mybir.dt.float32
I32 = mybir.dt.int32
I8 = mybir.dt.int8
Alu = mybir.AluOpType


def build_bfp_kernel(repeat=1):
    nc = bacc.Bacc("TRN2", target_bir_lowering=False, debug=False)
    x_d = nc.dram_tensor("x", [SH_ROWS, COLS], F32, kind="ExternalInput")
    o_d = nc.dram_tensor("out", [SH_ROWS, COLS], F32, kind="ExternalOutput")
    x_t = x_d.ap().rearrange("(t p) c -> t p c", p=TILE_P)
    o_t = o_d.ap().rearrange("(t p) c -> t p c", p=TILE_P)

    with TileContext(nc) as tc:
        with (
            tc.tile_pool(name="xp", bufs=3) as xp,
            tc.tile_pool(name="qp", bufs=3) as qp,
            tc.tile_pool(name="op", bufs=3) as op,
            tc.tile_pool(name="sp", bufs=3) as sp,
        ):
            for i in [t for _ in range(repeat) for t in range(N_TILES)]:
                xt = xp.tile([TILE_P, TILE_F], F32)
                nc.sync.dma_start(out=xt[:], in_=x_t[i])

                xtb = xt[:].rearrange("p (b k) -> p b k", k=BLK)
                mt = sp.tile([TILE_P, NB], F32, tag="m")
                nc.vector.tensor_reduce(
                    out=mt[:], in_=xtb, axis=mybir.AxisListType.X,
                    op=Alu.max, apply_absolute_value=True,
                )
                # step = (m & 0x7f800000) * 2^-7 ; recip = 2^(7-E) via exp flip
                st = sp.tile([TILE_P, NB], F32, tag="st")
                rt = sp.tile([TILE_P, NB], F32, tag="rt")
                nc.vector.tensor_scalar(
                    out=st[:].bitcast(I32), in0=mt[:].bitcast(I32),
                    scalar1=0x7F800000, scalar2=None, op0=Alu.bitwise_and,
                )
                nc.vector.tensor_scalar(
                    out=rt[:].bitcast(I32), in0=st[:].bitcast(I32),
                    scalar1=23, scalar2=None, op0=Alu.logical_shift_right,
                )
                nc.vector.tensor_scalar(
                    out=rt[:].bitcast(I32), in0=rt[:].bitcast(I32),
                    scalar1=-1, scalar2=261, op0=Alu.mult, op1=Alu.add,
                )
                nc.vector.tensor_scalar(
                    out=rt[:].bitcast(I32), in0=rt[:].bitcast(I32),
                    scalar1=23, scalar2=None, op0=Alu.logical_shift_left,
                )
                nc.vector.tensor_scalar(
                    out=st[:], in0=st[:], scalar1=float(2.0 ** -7),
                    scalar2=None, op0=Alu.mult,
                )

                # scale in-place: x *= recip  (DVE)
                rb = rt[:].unsqueeze(2).broadcast_to([TILE_P, NB, BLK])
                nc.vector.tensor_tensor(out=xtb, in0=xtb, in1=rb, op=Alu.mult)

                # round+clip via RNE+saturating convert (ACT)
                q8 = qp.tile([TILE_P, TILE_F], I8)
                nc.scalar.activation(
                    out=q8[:], in_=xt[:], func=mybir.ActivationFunctionType.Copy
                )

                # dequant: out = q8 * step  (GPSIMD)
                ot = op.tile([TILE_P, TILE_F], F32)
                sb = st[:].unsqueeze(2).broadcast_to([TILE_P, NB, BLK])
                nc.gpsimd.tensor_tensor(
                    out=ot[:].rearrange("p (b k) -> p b k", k=BLK),
                    in0=q8[:].rearrange("p (b k) -> p b k", k=BLK),
                    in1=sb, op=Alu.mult,
                )
                nc.sync.dma_start(out=o_t[i], in_=ot[:])

    nc.finalize()
    return nc


_NC_CACHE = {}


def _get_nc():
    if "nc" not in _NC_CACHE:
        _NC_CACHE["nc"] = build_bfp_kernel()
    return _NC_CACHE["nc"]


def kernel(x, mantissa_bits, block_size):
    assert int(mantissa_bits) == 8 and int(block_size) == 16
    x = np.ascontiguousarray(np.asarray(x, dtype=np.float32)).reshape(ROWS, COLS)
    nc = _get_nc()
    in_maps = [
        {"x": x[c * SH_ROWS:(c + 1) * SH_ROWS]} for c in range(N_CORES)
    ]
    res = run_bass_kernel_spmd(nc, in_maps, core_ids=list(range(N_CORES)))
    out = np.concatenate([r["out"] for r in res.results], axis=0)
    return out.reshape(FULL_SHAPE)
